# revision 14
# baseline (speedup 1.0000x reference)
"""Trainium2 Bass kernel for AttentionFlowLayer scores.

S[b,t,j] = C[b,t,:]@wC + Q[b,j,:]@wQ + sum_d C[b,t,d]*wCmQ[d]*Q[b,j,d] + bias

Full shapes: C [64,2048,128] f32, Q [64,512,128] f32 -> S [64,2048,512] f32.
Data-parallel over batch across 8 NeuronCores (8 batches per core).

Per core (software-pipelined over its 8 batches):
  - DMA C[b]/Q[b] into SBUF natural layout; PE-transpose 128x128 tiles to put
    d on partitions (fp32 PE transpose is exact). Transposes for batch b+1 are
    interleaved between batch b's matmul tiles so the PE never idles long
    enough for the HAM clock gate to re-throttle.
  - p1 folded into the main matmul: R[d,j] = Q^T[d,j]*wCmQ[d] + wC[d], so
    C_tile^T.T @ R = p3 + p1[t] (each row of R carries +wC[d]).
  - Main matmul in float32r (12-bit mantissa, 1 cyc/row when dense/warm).
    Modes: r1 = C_h@R_h (~1e-4 rel); q2 = + C_h@R_l (R to 24 bits, ~5e-5);
    r3 = + C_l@R_h (~1e-7).
  - p2+bias: exact 24-bit h/l rows. For ACT-epilogue tiles it is accumulated
    into PSUM by a K=2 f32r matmul (ones2.T @ [p2_h;p2_l]) and the epilogue
    is a plain ACT copy; for DVE-epilogue tiles the epilogue is a DVE
    tensor_tensor add against a replicated p2 tile. The split balances
    PE vs ACT vs DVE.
"""

import os
import sys

for _p in ("/opt/trn_rl_repo", "/opt/pypackages"):
    if _p not in sys.path and os.path.isdir(_p):
        sys.path.append(_p)

import numpy as np

import concourse.bass as bass
import concourse.mybir as mybir
import concourse.tile as tile
from concourse import bacc
from concourse.bass import ds, ts
from concourse.bass_utils import run_bass_kernel_spmd
from concourse.masks import make_identity

F32 = mybir.dt.float32
F32R = mybir.dt.float32r
AF = mybir.ActivationFunctionType
ALU = mybir.AluOpType

N_CORES = 8
B_FULL, T, D = 64, 2048, 128
J = 512
B_LOC = B_FULL // N_CORES  # 8 batches per core
N_TTILE = T // 128  # 16

MODE = os.environ.get("KERNEL_MODE", "r1")  # r1 | q2 | r3
# Tiles (of 16 per batch) using the aug-matmul + ACT-copy epilogue;
# the rest use the DVE tensor_tensor epilogue.
ACT_TILES = int(os.environ.get("KERNEL_ACT_TILES", "4"))


def _build_nc():
    nc = bacc.Bacc("TRN2", target_bir_lowering=False, debug=False,
                   num_devices=N_CORES)
    C_d = nc.dram_tensor("C_s", [B_LOC, T, D], F32, kind="ExternalInput")
    Q_d = nc.dram_tensor("Q_s", [B_LOC, J, D], F32, kind="ExternalInput")
    wc_d = nc.dram_tensor("wc_col", [128, 1], F32, kind="ExternalInput")
    wq_d = nc.dram_tensor("wq_col", [128, 1], F32, kind="ExternalInput")
    wcmq_d = nc.dram_tensor("wcmq_col", [128, 1], F32, kind="ExternalInput")
    bias_d = nc.dram_tensor("bias_rep", [128, 1], F32, kind="ExternalInput")
    wqo_d = nc.dram_tensor("wq_ones", [128, 128], F32, kind="ExternalInput")
    S_d = nc.dram_tensor("S_s", [B_LOC, T, J], F32, kind="ExternalOutput")

    r3 = MODE == "r3"
    q2 = MODE == "q2"

    import contextlib
    stack = contextlib.ExitStack()
    with tile.TileContext(nc) as tc, stack:
        const_pool = stack.enter_context(tc.tile_pool(name="const", bufs=1))
        cnat_pool = stack.enter_context(tc.tile_pool(name="cnat", bufs=3))
        qside_pool = stack.enter_context(tc.tile_pool(name="qside", bufs=3))
        ct_pool = stack.enter_context(tc.tile_pool(name="ct", bufs=2))
        out_pool = stack.enter_context(tc.tile_pool(name="outsb", bufs=3))
        ps_tr = stack.enter_context(tc.tile_pool(name="ps_tr", bufs=3,
                                                 space="PSUM"))
        ps_s = stack.enter_context(tc.tile_pool(name="ps_s", bufs=4,
                                                space="PSUM"))
        ps_p2 = stack.enter_context(tc.tile_pool(name="ps_p2", bufs=2,
                                                 space="PSUM"))

        ident = const_pool.tile([128, 128], F32, name="ident")
        make_identity(nc, ident[:])
        ones1_f = const_pool.tile([1, 128], F32, name="ones1_f")
        nc.vector.memset(ones1_f[:], 1.0)
        ones1 = const_pool.tile([1, 128], F32R, name="ones1")
        nc.vector.tensor_copy(ones1[:], ones1_f[:])
        wc_sb = const_pool.tile([128, 1], F32, name="wc_sb")
        nc.scalar.dma_start(wc_sb[:], wc_d.ap())
        wq_sb = const_pool.tile([128, 1], F32, name="wq_sb")
        nc.scalar.dma_start(wq_sb[:], wq_d.ap())
        wcmq_sb = const_pool.tile([128, 1], F32, name="wcmq_sb")
        nc.scalar.dma_start(wcmq_sb[:], wcmq_d.ap())
        bias_sb = const_pool.tile([128, 1], F32, name="bias_sb")
        nc.scalar.dma_start(bias_sb[:], bias_d.ap())
        wqo_sb = const_pool.tile([128, 128], F32, name="wqo_sb")
        nc.scalar.dma_start(wqo_sb[:], wqo_d.ap())
        wqo_r = const_pool.tile([128, 128], F32R, name="wqo_r")
        nc.vector.tensor_copy(wqo_r[:], wqo_sb[:])

        C_ap = C_d.ap()
        Q_ap = Q_d.ap()
        S_ap = S_d.ap()

        st = {}  # per-batch live tiles

        def emit_load(b):
            q_nat = qside_pool.tile([128, J], F32, name="q_nat", tag="q_nat")
            nc.sync.dma_start(
                q_nat[:].rearrange("p (n d) -> p n d", d=D),
                Q_ap[b].rearrange("(n p) d -> p n d", p=128))
            c_nat = cnat_pool.tile([128, T], F32, name="c_nat", tag="c_nat")
            # t = 16*p + k: each partition holds 16 consecutive t rows, so
            # the DRAM side is one 8KB-contiguous run per partition.
            cb = C_ap[b].rearrange("(p k) d -> p k d", k=16)
            for g in range(2):
                nc.sync.dma_start(
                    c_nat[:, ts(g, T // 2)].rearrange("p (k d) -> p k d", d=D),
                    cb[:, ds(8 * g, 8), :])
            st[b] = {"c_nat": c_nat, "q_nat": q_nat}

        def emit_qprep(b):
            s = st[b]
            trq = ps_tr.tile([128, J], F32, name="trq", tag="tr")
            for qi in range(J // 128):
                nc.tensor.transpose(trq[:, ts(qi, 128)],
                                    s["q_nat"][:, ts(qi, 128)], ident[:])
            qt = qside_pool.tile([128, J], F32R, name="qt", tag="qt")
            nc.scalar.activation(qt[:], trq[:], AF.Copy)

            r_full = qside_pool.tile([128, J], F32, name="r_full", tag="r_full")
            nc.vector.tensor_scalar(r_full[:], qt[:].bitcast(F32), wcmq_sb[:],
                                    wc_sb[:], ALU.mult, ALU.add)
            r_h = qside_pool.tile([128, J], F32R, name="r_h", tag="r_h")
            nc.gpsimd.tensor_copy(r_h[:], r_full[:])
            s["r_h"] = r_h
            if q2 or r3:
                r_l = qside_pool.tile([128, J], F32R, name="r_l", tag="r_l")
                nc.vector.tensor_sub(r_l[:], r_full[:], r_h[:].bitcast(F32))
                s["r_l"] = r_l

            # p2 replicated over partitions: (wQ outer ones) @ Q^T in f32r
            # (const weights -> no reload after first batch), +bias on copy.
            p2rps = ps_p2.tile([128, J], F32, name="p2rps", tag="p2rps", bufs=1)
            nc.tensor.matmul(p2rps[:], wqo_r[:], qt[:], start=True, stop=True)
            p2rep = qside_pool.tile([128, J], F32, name="p2rep", tag="p2rep")
            nc.scalar.activation(p2rep[:], p2rps[:], AF.Identity, bias=bias_sb[:])
            s["p2rep"] = p2rep
            if ACT_TILES > 0:
                p2row = qside_pool.tile([1, J], F32R, name="p2row", tag="p2row")
                nc.vector.tensor_copy(p2row[:], p2rep[0:1, :])
                s["p2row"] = p2row

        def emit_cprep_group(b, g):
            s = st[b]
            if "ct_h" not in s:
                s["ct_h"] = ct_pool.tile([128, T], F32R, name="ct_h", tag="ct_h")
                if r3:
                    s["ct_l"] = ct_pool.tile([128, T], F32R, name="ct_l",
                                             tag="ct_l")
            trp = ps_tr.tile([128, J], F32, name="trp", tag="tr")
            for k in range(4):
                i = 4 * g + k
                nc.tensor.transpose(trp[:, ts(k, 128)],
                                    s["c_nat"][:, ts(i, 128)], ident[:])
            nc.scalar.activation(s["ct_h"][:, ts(g, J)], trp[:], AF.Copy)
            if r3:
                nc.vector.tensor_sub(s["ct_l"][:, ts(g, J)], trp[:],
                                     s["ct_h"][:, ts(g, J)].bitcast(F32))

        def emit_tile(b, i):
            s = st[b]
            ct_h = s["ct_h"][:, ts(i, 128)]
            use_aug = i % N_TTILE < ACT_TILES
            sps = ps_s.tile([128, J], F32, name="sps", tag="sps")
            last_main = not use_aug
            nc.tensor.matmul(sps[:], ct_h[:], s["r_h"][:],
                             start=True, stop=last_main and not (q2 or r3))
            if q2 or r3:
                nc.tensor.matmul(sps[:], ct_h[:], s["r_l"][:],
                                 start=False, stop=last_main and not r3)
            if r3:
                nc.tensor.matmul(sps[:], s["ct_l"][:, ts(i, 128)], s["r_h"][:],
                                 start=False, stop=last_main)
            if i % 4 == 0:
                s["out4"] = out_pool.tile([128, 4 * J], F32, name="out4",
                                          tag="out4")
            out_sb = s["out4"][:, ts(i % 4, J)]
            if use_aug:
                nc.tensor.matmul(sps[:], ones1[:], s["p2row"][:],
                                 start=False, stop=True)
                nc.scalar.activation(out_sb[:], sps[:], AF.Copy)
            else:
                nc.vector.tensor_add(out_sb[:], sps[:], s["p2rep"][:])
            if i % 4 == 3:
                g = i // 4
                nc.scalar.dma_start(
                    S_ap[b].rearrange("(p k) j -> p k j", k=16)[
                        :, ds(4 * g, 4), :],
                    s["out4"][:].rearrange("p (k j) -> p k j", j=J))

        def emit_release(b):
            st.pop(b, None)

        # Software pipeline: prep for batch b+1 rides inside batch b's
        # matmul loop so the PE stream stays dense.
        emit_load(0)
        emit_load(1)
        emit_qprep(0)
        for g in range(4):
            emit_cprep_group(0, g)
        for b in range(B_LOC):
            for i in range(N_TTILE):
                emit_tile(b, i)
                if i == 0 and b + 2 < B_LOC:
                    emit_load(b + 2)
                if b + 1 < B_LOC:
                    if i == 1:
                        emit_qprep(b + 1)
                    elif i in (3, 7, 11, 15):
                        emit_cprep_group(b + 1, (i - 3) // 4)
            emit_release(b)

    nc.compile()
    return nc


_NC_CACHE = None


def _get_nc():
    global _NC_CACHE
    if _NC_CACHE is None:
        _NC_CACHE = _build_nc()
    return _NC_CACHE


def _make_in_maps(C, Q, weight_C, weight_Q, weight_CmQ, bias):
    C = np.ascontiguousarray(np.asarray(C, dtype=np.float32))
    Q = np.ascontiguousarray(np.asarray(Q, dtype=np.float32))
    wc = np.asarray(weight_C, dtype=np.float32).reshape(128, 1)
    wq = np.asarray(weight_Q, dtype=np.float32).reshape(128, 1)
    wcmq = np.asarray(weight_CmQ, dtype=np.float32).reshape(128, 1)
    bias_rep = np.full((128, 1), float(np.asarray(bias).reshape(-1)[0]),
                       dtype=np.float32)
    wq_ones = np.ascontiguousarray(np.tile(wq, (1, 128)))
    in_maps = []
    for k in range(N_CORES):
        in_maps.append({
            "C_s": np.ascontiguousarray(C[k * B_LOC:(k + 1) * B_LOC]),
            "Q_s": np.ascontiguousarray(Q[k * B_LOC:(k + 1) * B_LOC]),
            "wc_col": wc,
            "wq_col": wq,
            "wcmq_col": wcmq,
            "bias_rep": bias_rep,
            "wq_ones": wq_ones,
        })
    return in_maps


def _run(in_maps, **kw):
    nc = _get_nc()
    return run_bass_kernel_spmd(nc, in_maps, core_ids=list(range(N_CORES)), **kw)


def kernel(C, Q, weight_C, weight_Q, weight_CmQ, bias):
    in_maps = _make_in_maps(C, Q, weight_C, weight_Q, weight_CmQ, bias)
    res = _run(in_maps)
    return np.concatenate([r["S_s"] for r in res.results], axis=0)


def _install_ntff_hook():
    """Provide antenv.axon_hooks (absent on this image) backed by the
    libaxon_pjrt.so NRT-profile C ABI, so trace=True works under axon."""
    import types
    if "antenv.axon_hooks" in sys.modules:
        return
    try:
        from trn_agent_boot.trn_boot import _ntff_profile_via_ctypes
        hook = _ntff_profile_via_ctypes("/opt/axon/libaxon_pjrt.so")
    except Exception:
        hook = None
    mod = types.ModuleType("antenv.axon_hooks")
    _state = {"hook": hook}
    mod.set_axon_ntff_profile_hook = lambda h: _state.__setitem__("hook", h)
    mod.get_axon_ntff_profile_hook = lambda: _state["hook"]
    sys.modules["antenv.axon_hooks"] = mod


def kernel_traced(C, Q, weight_C, weight_Q, weight_CmQ, bias, **kw):
    """Like kernel() but with NTFF tracing; returns (out, BassKernelResults)."""
    _install_ntff_hook()
    in_maps = _make_in_maps(C, Q, weight_C, weight_Q, weight_CmQ, bias)
    res = _run(in_maps, trace=True, **kw)
    out = np.concatenate([r["S_s"] for r in res.results], axis=0)
    return out, res


# revision 15
# speedup vs baseline: 1.0714x; 1.0714x over previous
"""Trainium2 Bass kernel for AttentionFlowLayer scores.

S[b,t,j] = C[b,t,:]@wC + Q[b,j,:]@wQ + sum_d C[b,t,d]*wCmQ[d]*Q[b,j,d] + bias

Full shapes: C [64,2048,128] f32, Q [64,512,128] f32 -> S [64,2048,512] f32.
Data-parallel over batch across 8 NeuronCores (8 batches per core).

Per core (software-pipelined over its 8 batches):
  - DMA C[b]/Q[b] into SBUF natural layout; PE-transpose 128x128 tiles to put
    d on partitions (fp32 PE transpose is exact). Transposes for batch b+1 are
    interleaved between batch b's matmul tiles so the PE never idles long
    enough for the HAM clock gate to re-throttle.
  - p1 folded into the main matmul: R[d,j] = Q^T[d,j]*wCmQ[d] + wC[d], so
    C_tile^T.T @ R = p3 + p1[t] (each row of R carries +wC[d]).
  - Main matmul in float32r (12-bit mantissa, 1 cyc/row when dense/warm).
    Modes: r1 = C_h@R_h (~1e-4 rel); q2 = + C_h@R_l (R to 24 bits, ~5e-5);
    r3 = + C_l@R_h (~1e-7).
  - p2+bias: exact 24-bit h/l rows. For ACT-epilogue tiles it is accumulated
    into PSUM by a K=2 f32r matmul (ones2.T @ [p2_h;p2_l]) and the epilogue
    is a plain ACT copy; for DVE-epilogue tiles the epilogue is a DVE
    tensor_tensor add against a replicated p2 tile. The split balances
    PE vs ACT vs DVE.
"""

import os
import sys

for _p in ("/opt/trn_rl_repo", "/opt/pypackages"):
    if _p not in sys.path and os.path.isdir(_p):
        sys.path.append(_p)

import numpy as np

import concourse.bass as bass
import concourse.mybir as mybir
import concourse.tile as tile
from concourse import bacc
from concourse.bass import ds, ts
from concourse.bass_utils import run_bass_kernel_spmd
from concourse.masks import make_identity

F32 = mybir.dt.float32
F32R = mybir.dt.float32r
AF = mybir.ActivationFunctionType
ALU = mybir.AluOpType

N_CORES = 8
B_FULL, T, D = 64, 2048, 128
J = 512
B_LOC = B_FULL // N_CORES  # 8 batches per core
N_TTILE = T // 128  # 16

MODE = os.environ.get("KERNEL_MODE", "r1")  # r1 | q2 | r3
# Tiles (of 16 per batch) using the aug-matmul + ACT-copy epilogue;
# the rest use the DVE tensor_tensor epilogue.
ACT_TILES = int(os.environ.get("KERNEL_ACT_TILES", "4"))


def _build_nc():
    nc = bacc.Bacc("TRN2", target_bir_lowering=False, debug=False,
                   num_devices=N_CORES)
    C_d = nc.dram_tensor("C_s", [B_LOC, T, D], F32, kind="ExternalInput")
    Q_d = nc.dram_tensor("Q_s", [B_LOC, J, D], F32, kind="ExternalInput")
    wc_d = nc.dram_tensor("wc_col", [128, 1], F32, kind="ExternalInput")
    wq_d = nc.dram_tensor("wq_col", [128, 1], F32, kind="ExternalInput")
    wcmq_d = nc.dram_tensor("wcmq_col", [128, 1], F32, kind="ExternalInput")
    bias_d = nc.dram_tensor("bias_rep", [128, 1], F32, kind="ExternalInput")
    wqo_d = nc.dram_tensor("wq_ones", [128, 128], F32, kind="ExternalInput")
    S_d = nc.dram_tensor("S_s", [B_LOC, T, J], F32, kind="ExternalOutput")

    r3 = MODE == "r3"
    q2 = MODE == "q2"

    import contextlib
    stack = contextlib.ExitStack()
    with tile.TileContext(nc) as tc, stack:
        const_pool = stack.enter_context(tc.tile_pool(name="const", bufs=1))
        cnat_pool = stack.enter_context(tc.tile_pool(name="cnat", bufs=3))
        qside_pool = stack.enter_context(tc.tile_pool(name="qside", bufs=3))
        ct_pool = stack.enter_context(tc.tile_pool(name="ct", bufs=2))
        out_pool = stack.enter_context(tc.tile_pool(name="outsb", bufs=3))
        ps_tr = stack.enter_context(tc.tile_pool(name="ps_tr", bufs=3,
                                                 space="PSUM"))
        ps_s = stack.enter_context(tc.tile_pool(name="ps_s", bufs=4,
                                                space="PSUM"))
        ps_p2 = stack.enter_context(tc.tile_pool(name="ps_p2", bufs=2,
                                                 space="PSUM"))

        ident = const_pool.tile([128, 128], F32, name="ident")
        make_identity(nc, ident[:])
        ones1_f = const_pool.tile([1, 128], F32, name="ones1_f")
        nc.vector.memset(ones1_f[:], 1.0)
        ones1 = const_pool.tile([1, 128], F32R, name="ones1")
        nc.vector.tensor_copy(ones1[:], ones1_f[:])
        wc_sb = const_pool.tile([128, 1], F32, name="wc_sb")
        nc.scalar.dma_start(wc_sb[:], wc_d.ap())
        wq_sb = const_pool.tile([128, 1], F32, name="wq_sb")
        nc.scalar.dma_start(wq_sb[:], wq_d.ap())
        wcmq_sb = const_pool.tile([128, 1], F32, name="wcmq_sb")
        nc.scalar.dma_start(wcmq_sb[:], wcmq_d.ap())
        bias_sb = const_pool.tile([128, 1], F32, name="bias_sb")
        nc.scalar.dma_start(bias_sb[:], bias_d.ap())
        wqo_sb = const_pool.tile([128, 128], F32, name="wqo_sb")
        nc.scalar.dma_start(wqo_sb[:], wqo_d.ap())
        wqo_r = const_pool.tile([128, 128], F32R, name="wqo_r")
        nc.vector.tensor_copy(wqo_r[:], wqo_sb[:])

        C_ap = C_d.ap()
        Q_ap = Q_d.ap()
        S_ap = S_d.ap()

        st = {}  # per-batch live tiles

        def emit_load(b):
            q_nat = qside_pool.tile([128, J], F32, name="q_nat", tag="q_nat")
            nc.sync.dma_start(
                q_nat[:].rearrange("p (n d) -> p n d", d=D),
                Q_ap[b].rearrange("(n p) d -> p n d", p=128))
            c_nat = cnat_pool.tile([128, T], F32, name="c_nat", tag="c_nat")
            # t = 16*p + k: each partition holds 16 consecutive t rows, so
            # the DRAM side is one 8KB-contiguous run per partition.
            cb = C_ap[b].rearrange("(p k) d -> p k d", k=16)
            for g in range(2):
                nc.sync.dma_start(
                    c_nat[:, ts(g, T // 2)].rearrange("p (k d) -> p k d", d=D),
                    cb[:, ds(8 * g, 8), :])
            st[b] = {"c_nat": c_nat, "q_nat": q_nat}

        def emit_qprep(b):
            s = st[b]
            trq = ps_tr.tile([128, J], F32, name="trq", tag="tr")
            for qi in range(J // 128):
                nc.tensor.transpose(trq[:, ts(qi, 128)],
                                    s["q_nat"][:, ts(qi, 128)], ident[:])
            qt = qside_pool.tile([128, J], F32R, name="qt", tag="qt")
            nc.scalar.activation(qt[:], trq[:], AF.Copy)

            r_full = qside_pool.tile([128, J], F32, name="r_full", tag="r_full")
            nc.vector.tensor_scalar(r_full[:], qt[:].bitcast(F32), wcmq_sb[:],
                                    wc_sb[:], ALU.mult, ALU.add)
            r_h = qside_pool.tile([128, J], F32R, name="r_h", tag="r_h")
            nc.gpsimd.tensor_copy(r_h[:], r_full[:])
            s["r_h"] = r_h
            if q2 or r3:
                r_l = qside_pool.tile([128, J], F32R, name="r_l", tag="r_l")
                nc.vector.tensor_sub(r_l[:], r_full[:], r_h[:].bitcast(F32))
                s["r_l"] = r_l

            # p2 replicated over partitions: (wQ outer ones) @ Q^T in f32r
            # (const weights -> no reload after first batch), +bias on copy.
            p2rps = ps_p2.tile([128, J], F32, name="p2rps", tag="p2rps", bufs=1)
            nc.tensor.matmul(p2rps[:], wqo_r[:], qt[:], start=True, stop=True)
            p2rep = qside_pool.tile([128, J], F32, name="p2rep", tag="p2rep")
            nc.vector.tensor_scalar_add(p2rep[:], p2rps[:], bias_sb[:])
            s["p2rep"] = p2rep
            if ACT_TILES > 0:
                p2row = qside_pool.tile([1, J], F32R, name="p2row", tag="p2row")
                nc.vector.tensor_copy(p2row[:], p2rep[0:1, :])
                s["p2row"] = p2row

        def emit_cprep_group(b, g):
            s = st[b]
            if "ct_h" not in s:
                s["ct_h"] = ct_pool.tile([128, T], F32R, name="ct_h", tag="ct_h")
                if r3:
                    s["ct_l"] = ct_pool.tile([128, T], F32R, name="ct_l",
                                             tag="ct_l")
            trp = ps_tr.tile([128, J], F32, name="trp", tag="tr")
            for k in range(4):
                i = 4 * g + k
                nc.tensor.transpose(trp[:, ts(k, 128)],
                                    s["c_nat"][:, ts(i, 128)], ident[:])
            nc.scalar.activation(s["ct_h"][:, ts(g, J)], trp[:], AF.Copy)
            if r3:
                nc.vector.tensor_sub(s["ct_l"][:, ts(g, J)], trp[:],
                                     s["ct_h"][:, ts(g, J)].bitcast(F32))

        def emit_tile(b, i):
            s = st[b]
            ct_h = s["ct_h"][:, ts(i, 128)]
            use_aug = i % N_TTILE < ACT_TILES
            sps = ps_s.tile([128, J], F32, name="sps", tag="sps")
            last_main = not use_aug
            nc.tensor.matmul(sps[:], ct_h[:], s["r_h"][:],
                             start=True, stop=last_main and not (q2 or r3))
            if q2 or r3:
                nc.tensor.matmul(sps[:], ct_h[:], s["r_l"][:],
                                 start=False, stop=last_main and not r3)
            if r3:
                nc.tensor.matmul(sps[:], s["ct_l"][:, ts(i, 128)], s["r_h"][:],
                                 start=False, stop=last_main)
            if i % 4 == 0:
                s["out4"] = out_pool.tile([128, 4 * J], F32, name="out4",
                                          tag="out4")
            out_sb = s["out4"][:, ts(i % 4, J)]
            if use_aug:
                nc.tensor.matmul(sps[:], ones1[:], s["p2row"][:],
                                 start=False, stop=True)
                nc.scalar.activation(out_sb[:], sps[:], AF.Copy)
            else:
                nc.vector.tensor_add(out_sb[:], sps[:], s["p2rep"][:])
            if i % 4 == 3:
                g = i // 4
                nc.scalar.dma_start(
                    S_ap[b].rearrange("(p k) j -> p k j", k=16)[
                        :, ds(4 * g, 4), :],
                    s["out4"][:].rearrange("p (k j) -> p k j", j=J))

        def emit_release(b):
            st.pop(b, None)

        # Software pipeline: prep for batch b+1 rides inside batch b's
        # matmul loop so the PE stream stays dense.
        emit_load(0)
        emit_load(1)
        emit_qprep(0)
        for g in range(4):
            emit_cprep_group(0, g)
        for b in range(B_LOC):
            for i in range(N_TTILE):
                emit_tile(b, i)
                if i == 0 and b + 2 < B_LOC:
                    emit_load(b + 2)
                if b + 1 < B_LOC:
                    if i == 1:
                        emit_qprep(b + 1)
                    elif i in (3, 7, 11, 15):
                        emit_cprep_group(b + 1, (i - 3) // 4)
            emit_release(b)

    nc.compile()
    return nc


_NC_CACHE = None


def _get_nc():
    global _NC_CACHE
    if _NC_CACHE is None:
        _NC_CACHE = _build_nc()
    return _NC_CACHE


def _make_in_maps(C, Q, weight_C, weight_Q, weight_CmQ, bias):
    C = np.ascontiguousarray(np.asarray(C, dtype=np.float32))
    Q = np.ascontiguousarray(np.asarray(Q, dtype=np.float32))
    wc = np.asarray(weight_C, dtype=np.float32).reshape(128, 1)
    wq = np.asarray(weight_Q, dtype=np.float32).reshape(128, 1)
    wcmq = np.asarray(weight_CmQ, dtype=np.float32).reshape(128, 1)
    bias_rep = np.full((128, 1), float(np.asarray(bias).reshape(-1)[0]),
                       dtype=np.float32)
    wq_ones = np.ascontiguousarray(np.tile(wq, (1, 128)))
    in_maps = []
    for k in range(N_CORES):
        in_maps.append({
            "C_s": np.ascontiguousarray(C[k * B_LOC:(k + 1) * B_LOC]),
            "Q_s": np.ascontiguousarray(Q[k * B_LOC:(k + 1) * B_LOC]),
            "wc_col": wc,
            "wq_col": wq,
            "wcmq_col": wcmq,
            "bias_rep": bias_rep,
            "wq_ones": wq_ones,
        })
    return in_maps


def _run(in_maps, **kw):
    nc = _get_nc()
    return run_bass_kernel_spmd(nc, in_maps, core_ids=list(range(N_CORES)), **kw)


def kernel(C, Q, weight_C, weight_Q, weight_CmQ, bias):
    in_maps = _make_in_maps(C, Q, weight_C, weight_Q, weight_CmQ, bias)
    res = _run(in_maps)
    return np.concatenate([r["S_s"] for r in res.results], axis=0)


def _install_ntff_hook():
    """Provide antenv.axon_hooks (absent on this image) backed by the
    libaxon_pjrt.so NRT-profile C ABI, so trace=True works under axon."""
    import types
    if "antenv.axon_hooks" in sys.modules:
        return
    try:
        from trn_agent_boot.trn_boot import _ntff_profile_via_ctypes
        hook = _ntff_profile_via_ctypes("/opt/axon/libaxon_pjrt.so")
    except Exception:
        hook = None
    mod = types.ModuleType("antenv.axon_hooks")
    _state = {"hook": hook}
    mod.set_axon_ntff_profile_hook = lambda h: _state.__setitem__("hook", h)
    mod.get_axon_ntff_profile_hook = lambda: _state["hook"]
    sys.modules["antenv.axon_hooks"] = mod


def kernel_traced(C, Q, weight_C, weight_Q, weight_CmQ, bias, **kw):
    """Like kernel() but with NTFF tracing; returns (out, BassKernelResults)."""
    _install_ntff_hook()
    in_maps = _make_in_maps(C, Q, weight_C, weight_Q, weight_CmQ, bias)
    res = _run(in_maps, trace=True, **kw)
    out = np.concatenate([r["S_s"] for r in res.results], axis=0)
    return out, res
